# revision 1
# baseline (speedup 1.0000x reference)
"""CT parallel-beam 2D forward projector on 8 Trainium2 NeuronCores.

Algorithm (exact, validated vs reference to ~1.6e-5 rel err):
  For each view angle, the trapezoid-footprint bin weights are written via the
  trapezoid CDF  Phic(t) = q*[relu^2(t) - relu^2(t-B) - relu^2(t-A) + relu^2(t-A-B)]
  (A = max(|cos|,sin), B = min, q = 1/(2AB)).  With the separable floor split
  z = p_xi(xi) + p_eta(eta),  b_xi = floor(p_xi), b_eta = floor(p_eta),
  g = frac_xi + frac_eta in [0,2), every pixel scatters into bins
  n = b_xi + b_eta + j (j = 0..3) with weights U_j(g) = Phi_{j+1}(g) - Phi_j(g),
  Phi_i(g) = Phic(i - 1/2 - g), Phi_0 = 0, Phi_4 = 1 (the floor carry is absorbed
  by the continuous U_j).  Abel summation turns the 4 tap fields into gathers of
  T_i = img * Phi_i (i = 1..3) and img itself at slots i-1 (+) and i (-).

  Device pipeline per (angle, 128-row eta-chunk), layout [eta-part, xi-free]:
    ACT : y_t = Relu(-f_xi + (E_t - f_eta))  for 12 constants E_t, r_t = y_t^2
    DVE : Phi_i combine, T_i = (s*q)*img, plus run-sum S halves
    GPSIMD: indirect_copy gathers (monotone xi->bin binning, host-built indices)
    PE  : one-hot matmul over eta (local per-chunk bins v' < 96), PSUM-accumulated
          over the 7 signed gather instances
  Host: tiny anti-diagonal collapse R[v',m] -> proj[n], plus direct numpy path for
  the two degenerate axis-aligned angles (B ~ 0).

SPMD: one program for all 8 cores. Cores 0-3 process "class X" angles
(|cos| >= sin) on img; cores 4-7 process "class Y" angles on img.T. All
per-angle variation (tables, one-hots, gather indices) is input data.
"""

import numpy as np

Nx = Ny = 512
Nu = 768
NTHETA = 180
HALF_U = (Nu - 1) / 2.0
NCORES = 8
import os as _os
APC = int(_os.environ.get("CT_APC", "23"))   # angles per core
NCHUNK = 4        # eta chunks of 128
MPAD = 528        # gather output width (W <= 513, padded, mult of 16)
RPAD = 544        # R output width (W + 3 <= 516, plus pad)
PS1W = 32         # second PSUM piece width (covers m in [512, 531))
VP = 96           # local v' bins per chunk (128*0.7072 < 91)
ZERO_COL = 1023   # index of the all-zero column in each C buffer
B_RECT = 1e-4     # below this min-slope, use the host rect path

_PROGRAM_CACHE = {}


# --------------------------------------------------------------------------
# host tables
# --------------------------------------------------------------------------

def _angle_tables(theta_val):
    th = float(theta_val)
    c, s = np.cos(th), np.sin(th)
    ac, asn = abs(c), abs(s)
    A, B = max(ac, asn), min(ac, asn)
    b2 = ac + asn
    cls = 0 if ac >= asn else 1
    a_xi, a_eta = (c, s) if cls == 0 else (s, c)
    z0 = HALF_U - b2 / 2 - 255.5 * (c + s)
    grid = np.arange(512)
    pxi = a_xi * grid + z0
    peta = a_eta * grid
    bxi = np.floor(pxi).astype(np.int64)
    fxi = pxi - bxi
    beta = np.floor(peta).astype(np.int64)
    feta = peta - beta
    q = 1.0 / (2 * A * B) if B > B_RECT else None
    return dict(c=c, s=s, A=A, B=B, b2=b2, q=q, cls=cls,
                bxi=bxi, fxi=fxi, beta=beta, feta=feta)


def _gather_tables(T):
    """xi-binning run-starts and the 7 instance index streams (length MPAD)."""
    bxi = T["bxi"]
    bxi_min = int(bxi.min())
    mloc = bxi - bxi_min
    W = int(mloc.max()) + 1
    # run start xa[m] and length L[m] (1 or 2) for each bin m
    xa = np.zeros(W, dtype=np.int64)
    L = np.zeros(W, dtype=np.int64)
    order = np.argsort(mloc, kind="stable")
    sorted_m = mloc[order]
    first = np.searchsorted(sorted_m, np.arange(W), side="left")
    last = np.searchsorted(sorted_m, np.arange(W), side="right")
    for m in range(W):
        idxs = order[first[m]:last[m]]
        n = len(idxs)
        assert 1 <= n <= 2
        xa[m] = idxs.min()
        L[m] = n
        if n == 2:
            assert idxs.max() - idxs.min() == 1

    # single zero-shift stream; slot shifts are applied as PSUM column offsets
    idx = np.full(MPAD, ZERO_COL, dtype=np.int64)
    msrc = np.arange(0, min(W, MPAD))
    idx[:len(msrc)] = np.where(L[msrc] == 2, 512 + xa[msrc], xa[msrc])
    return dict(bxi_min=bxi_min, W=W, stream=idx)


def _wrap_idx(stream):
    """[MPAD] int -> [128, MPAD//16] uint16 wrapped per 16-partition groups."""
    w = stream.reshape(MPAD // 16, 16).T.astype(np.uint16)   # [16, MPAD/16]
    return np.tile(w, (8, 1))                                 # [128, MPAD/16]


def _core_inputs(img_layout, angle_list, tables):
    """Build the input map for one core. img_layout: [512,512] f32 in [eta,xi]."""
    A_ = APC
    fxi_t = np.zeros((A_, 512), dtype=np.float32)
    bias_t = np.zeros((A_, NCHUNK, 128, 16), dtype=np.float32)
    oh_t = np.zeros((A_, NCHUNK, 128, VP), dtype=np.float32)
    idx_t = np.zeros((A_, 128, MPAD // 16), dtype=np.uint16)
    meta = []
    for ai, a in enumerate(angle_list):
        T = tables[a]
        G = _gather_tables(T)
        fxi_t[ai] = T["fxi"].astype(np.float32)
        knots = [0.0, T["B"], T["A"], T["A"] + T["B"]]
        feta = T["feta"]
        beta = T["beta"]
        for k in range(NCHUNK):
            sl = slice(k * 128, (k + 1) * 128)
            col = 0
            for i in (1, 2, 3):
                for kn in knots:
                    E = i - 0.5 - kn
                    bias_t[ai, k, :, col] = (E - feta[sl]).astype(np.float32)
                    col += 1
            bias_t[ai, k, :, 12] = np.float32(T["q"])
            bias_t[ai, k, :, 13] = feta[sl].astype(np.float32)
            vloc = beta[sl] - beta[sl].min()
            assert vloc.min() >= 0 and vloc.max() < VP, (vloc.min(), vloc.max())
            oh_t[ai, k, np.arange(128), vloc] = 1.0
        idx_t[ai] = _wrap_idx(G["stream"])
        meta.append(dict(angle=a, bxi_min=G["bxi_min"], W=G["W"],
                         beta0=[int(beta[k * 128:(k + 1) * 128].min())
                                for k in range(NCHUNK)]))
    in_map = {
        "imgL": np.ascontiguousarray(img_layout).astype(np.float32),
        "fxi_t": fxi_t,
        "bias_t": bias_t,
        "oh_t": oh_t,
        "idx_t": idx_t,
    }
    return in_map, meta


# --------------------------------------------------------------------------
# the bass program (identical for all cores)
# --------------------------------------------------------------------------

def _build_program():
    if "nc" in _PROGRAM_CACHE:
        return _PROGRAM_CACHE["nc"], _PROGRAM_CACHE["io"]

    import concourse.bass as bass
    import concourse.tile as tile
    from concourse import bacc, mybir
    from contextlib import ExitStack

    dt = mybir.dt
    AF = mybir.ActivationFunctionType
    ALU = mybir.AluOpType

    # engine assignment config. A=ACT, D=DVE, G=GPSIMD.
    # iform: per-i pipeline form (A = ACT relu+square, D = DVE min+products)
    cfg_s = _os.environ.get(
        "CT_CFG",
        "iform=AAD;sq=AAAAAAAAAAAA;comb=DDDDDDDDD;dcomb=DDDDDDD;ts=D;"
        "shalf=DDDD;imgcopy=A;drain=A")  # best of TimelineSim sweep (1.25 ms)
    CFG = dict(kv.split("=") for kv in cfg_s.split(";"))
    _PROGRAM_CACHE["cfg"] = CFG

    nc = bacc.Bacc("TRN2", target_bir_lowering=False, debug=False,
                   num_devices=NCORES)

    imgL = nc.dram_tensor("imgL", [512, 512], dt.float32, kind="ExternalInput").ap()
    fxi_t = nc.dram_tensor("fxi_t", [APC, 512], dt.float32, kind="ExternalInput").ap()
    bias_t = nc.dram_tensor("bias_t", [APC, NCHUNK, 128, 16], dt.float32,
                            kind="ExternalInput").ap()
    oh_t = nc.dram_tensor("oh_t", [APC, NCHUNK, 128, VP], dt.float32,
                          kind="ExternalInput").ap()
    idx_t = nc.dram_tensor("idx_t", [APC, 128, MPAD // 16], dt.uint16,
                           kind="ExternalInput").ap()
    r_out = nc.dram_tensor("r_out", [APC, NCHUNK, VP, RPAD], dt.float32,
                           kind="ExternalOutput").ap()

    # (field, psum column shift, sign); order chosen so the first writer of
    # each PSUM tile covers its full written range (start=True coverage)
    instances = [(0, 0, +1), (3, 3, +1), (1, 1, +1), (2, 2, +1),
                 (0, 1, -1), (1, 2, -1), (2, 3, -1)]

    with tile.TileContext(nc) as tc, ExitStack() as ctx:
        BB = int(_os.environ.get("CT_BUFS", "0"))  # 1 = bigger pools
        img_pool = ctx.enter_context(tc.tile_pool(name="img", bufs=1))
        row_pool = ctx.enter_context(tc.tile_pool(name="rows", bufs=2))
        tab_pool = ctx.enter_context(tc.tile_pool(name="tabs", bufs=2 + BB))
        y_pool = ctx.enter_context(tc.tile_pool(name="ys", bufs=3 + BB))
        r_pool = ctx.enter_context(tc.tile_pool(name="rs", bufs=3 + BB))
        ph_pool = ctx.enter_context(tc.tile_pool(name="phi", bufs=2 + BB))
        c_pool = ctx.enter_context(tc.tile_pool(name="cbuf", bufs=2 + BB))
        g_pool = ctx.enter_context(tc.tile_pool(name="gath", bufs=2 + BB))
        ps_pool = ctx.enter_context(tc.tile_pool(name="psum", bufs=2, space="PSUM"))
        o_pool = ctx.enter_context(tc.tile_pool(name="outs", bufs=2 + BB))

        # resident image chunks
        img_ch = []
        for k in range(NCHUNK):
            t = img_pool.tile([128, 512], dt.float32, tag=f"imgc{k}")
            nc.sync.dma_start(t[:], imgL[k * 128:(k + 1) * 128, :])
            img_ch.append(t)

        for ai in range(APC):
            fxi_bt = row_pool.tile([128, 512], dt.float32, tag="fxib")
            nc.sync.dma_start(fxi_bt[:],
                              fxi_t[ai:ai + 1, :].to_broadcast([128, 512]))
            fxi_b = fxi_bt[:]

            idxt = tab_pool.tile([128, MPAD // 16], dt.uint16, tag="idx")
            nc.sync.dma_start(idxt[:], idx_t[ai])

            for k in range(NCHUNK):
                bias = tab_pool.tile([128, 16], dt.float32, tag="bias")
                nc.sync.dma_start(bias[:], bias_t[ai, k])
                oh = tab_pool.tile([128, VP], dt.float32, tag="oh")
                nc.sync.dma_start(oh[:], oh_t[ai, k])
                ohn = tab_pool.tile([128, VP], dt.float32, tag="ohn")
                nc.vector.tensor_scalar(ohn[:], oh[:], -1.0, None, ALU.mult)

                qAP = bias[:, 12:13]

                # C buffers: [F(512) | S(511) | pad | zero col]
                cbufs = []
                for f in range(4):
                    cb = c_pool.tile([128, 1024], dt.float32, tag=f"c{f}")
                    cbufs.append(cb)

                # engine helpers for load balancing (cfg chars: A/D/G)
                def eng(ch):
                    return {"A": nc.scalar, "D": nc.vector, "G": nc.gpsimd}[ch]

                def tt(ch, out, a, b, op):
                    if ch == "A":
                        ch = "D"  # ACT has no tensor_tensor
                    eng(ch).tensor_tensor(out, a, b, op)

                # img field straight into C3
                if CFG["imgcopy"] == "A":
                    nc.scalar.copy(cbufs[3][:, 0:512], img_ch[k][:])
                else:
                    eng(CFG["imgcopy"]).tensor_copy(cbufs[3][:, 0:512], img_ch[k][:])

                # --- 12 relu^2 terms + Phi combine (two alternative forms)
                for i in range(3):
                    form = CFG["iform"][i]
                    if form in ("A", "H"):
                        ys = []
                        for kn in range(4):
                            y = y_pool.tile([128, 512], dt.float32, tag=f"y{kn}")
                            if form == "A":
                                nc.scalar.activation(
                                    y[:], fxi_b, AF.Relu,
                                    bias=bias[:, 4 * i + kn:4 * i + kn + 1],
                                    scale=-1.0)
                            else:
                                # y' = min(g-E, 0) = -relu(E-g); y'^2 == relu^2
                                eng(CFG["ts"]).tensor_scalar(
                                    y[:], fxi_b,
                                    bias[:, 4 * i + kn:4 * i + kn + 1],
                                    0.0, ALU.subtract, ALU.min)
                            ys.append(y)
                        rs = []
                        for kn in range(4):
                            t = 4 * i + kn
                            r = r_pool.tile([128, 512], dt.float32, tag=f"r{kn}")
                            ch = CFG["sq"][t]
                            if ch == "A":
                                nc.scalar.activation(r[:], ys[kn][:], AF.Square)
                            else:
                                eng(ch).tensor_tensor(r[:], ys[kn][:], ys[kn][:],
                                                      ALU.mult)
                            rs.append(r)
                        s12 = ph_pool.tile([128, 512], dt.float32, tag="s12")
                        tt(CFG["comb"][3 * i + 0], s12[:], rs[0][:], rs[1][:],
                           ALU.subtract)
                        s34 = ph_pool.tile([128, 512], dt.float32, tag="s34")
                        tt(CFG["comb"][3 * i + 1], s34[:], rs[3][:], rs[2][:],
                           ALU.subtract)
                        ssum = ph_pool.tile([128, 512], dt.float32, tag="ssum")
                        tt(CFG["comb"][3 * i + 2], ssum[:], s12[:], s34[:],
                           ALU.add)
                    else:
                        # y'_kn = min(g - E, 0) = -relu(E - g); r = y'^2
                        # r1-r2 = (y1'-y2')(y1'+y2'), r4-r3 = (y4'-y3')(y4'+y3')
                        ys = []
                        for kn in range(4):
                            y = y_pool.tile([128, 512], dt.float32, tag=f"y{kn}")
                            eng(CFG["ts"]).tensor_scalar(
                                y[:], fxi_b, bias[:, 4 * i + kn:4 * i + kn + 1],
                                0.0, ALU.subtract, ALU.min)
                            ys.append(y)
                        d12 = ph_pool.tile([128, 512], dt.float32, tag="s12")
                        tt(CFG["dcomb"][0], d12[:], ys[0][:], ys[1][:],
                           ALU.subtract)
                        a12 = r_pool.tile([128, 512], dt.float32, tag="r0")
                        tt(CFG["dcomb"][1], a12[:], ys[0][:], ys[1][:], ALU.add)
                        m12 = r_pool.tile([128, 512], dt.float32, tag="r1")
                        tt(CFG["dcomb"][2], m12[:], d12[:], a12[:], ALU.mult)
                        d34 = ph_pool.tile([128, 512], dt.float32, tag="s34")
                        tt(CFG["dcomb"][3], d34[:], ys[3][:], ys[2][:],
                           ALU.subtract)
                        a34 = r_pool.tile([128, 512], dt.float32, tag="r2")
                        tt(CFG["dcomb"][4], a34[:], ys[3][:], ys[2][:], ALU.add)
                        m34 = r_pool.tile([128, 512], dt.float32, tag="r3")
                        tt(CFG["dcomb"][5], m34[:], d34[:], a34[:], ALU.mult)
                        ssum = ph_pool.tile([128, 512], dt.float32, tag="ssum")
                        tt(CFG["dcomb"][6], ssum[:], m12[:], m34[:], ALU.add)
                    # T_i = (ssum * q) * img  -> C_i F-half
                    nc.vector.scalar_tensor_tensor(
                        cbufs[i][:, 0:512], ssum[:], qAP, img_ch[k][:],
                        ALU.mult, ALU.mult)

                # S halves: C[:, 512:1023] = F[:, 0:511] + F[:, 1:512]
                for f in range(4):
                    tt(CFG["shalf"][f], cbufs[f][:, 512:1023],
                       cbufs[f][:, 0:511], cbufs[f][:, 1:512], ALU.add)
                    nc.vector.memset(cbufs[f][:, 1023:1024], 0.0)

                # --- 4 gathers (T1, T2, T3, IMG), one shared index stream
                gts = []
                for f in range(4):
                    gt = g_pool.tile([128, MPAD], dt.float32, tag=f"g{f}")
                    nc.gpsimd.indirect_copy(gt[:], cbufs[f][:], idxt[:], True)
                    gts.append(gt)

                # --- PE one-hot eta-binning; slot shifts via PSUM col offsets
                # ps0 covers output m in [0, 512); ps1 covers [512, 531)
                ps0 = ps_pool.tile([VP, 512], dt.float32, tag="ps0")
                ps1 = ps_pool.tile([VP, PS1W], dt.float32, tag="ps1")
                mms = []  # (tile_id, out_ap, lhs, rhs_ap)
                for f, s, sgn in instances:
                    lhs = oh if sgn > 0 else ohn
                    mms.append((0, ps0[:, s:512], lhs, gts[f][:, 0:512 - s]))
                    mms.append((1, ps1[:, 0:16 + s], lhs,
                                gts[f][:, 512 - s:528]))
                # order: first full-coverage writer per tile, then the rest
                order = [0, 3, 1, 2] + list(range(4, 14))
                started = {0: False, 1: False}
                for pos, mi in enumerate(order):
                    tid, out_ap, lhs, rhs_ap = mms[mi]
                    is_first = not started[tid]
                    started[tid] = True
                    is_last = (pos == max(p for p, m in enumerate(order)
                                          if mms[m][0] == tid))
                    nc.tensor.matmul(out_ap, lhs[:], rhs_ap,
                                     start=is_first, stop=is_last)

                rout = o_pool.tile([VP, RPAD], dt.float32, tag="rout")
                if CFG["drain"] == "A":
                    nc.scalar.copy(rout[:, 0:512], ps0[:])
                    nc.scalar.copy(rout[:, 512:531], ps1[:, 0:19])
                else:
                    nc.vector.tensor_copy(rout[:, 0:512], ps0[:])
                    nc.vector.tensor_copy(rout[:, 512:531], ps1[:, 0:19])
                nc.vector.memset(rout[:, 531:RPAD], 0.0)
                nc.sync.dma_start(r_out[ai, k][:, 0:RPAD], rout[:])

    nc.compile()
    _PROGRAM_CACHE["nc"] = nc
    _PROGRAM_CACHE["io"] = None
    return nc, None


# --------------------------------------------------------------------------
# host-side rect path (degenerate angles) — numpy port of the reference
# --------------------------------------------------------------------------

def _host_project(img, theta_vals):
    y = (np.arange(Ny) - (Ny - 1) / 2.0)
    x = (np.arange(Nx) - (Nx - 1) / 2.0)
    y2d, x2d = np.meshgrid(y, x, indexing="ij")
    img_v = img.reshape(-1).astype(np.float64)
    out = np.zeros((len(theta_vals), Nu), dtype=np.float64)
    K = 4
    for t, th in enumerate(theta_vals):
        th = float(th)
        cos_t, sin_t = np.cos(th), np.sin(th)
        ac, asn = abs(cos_t), abs(sin_t)
        h = min(1.0 / ac if ac > 0 else np.inf, 1.0 / asn if asn > 0 else np.inf)
        b1 = abs(asn - ac)
        b2 = abs(asn + ac)
        u0 = x2d * cos_t + y2d * sin_t
        u1 = u0 - b2 / 2
        u2 = u0 - b1 / 2
        u3 = u0 + b1 / 2
        u4 = u0 + b2 / 2
        base = np.floor(u1 + HALF_U).astype(np.int64)
        den12 = (u2 - u1) + (u1 == u2)
        den34 = (u4 - u3) + (u3 == u4)
        acc = np.zeros(Nu + 8, dtype=np.float64)
        for k in range(K):
            idx = base + k
            u = idx - HALF_U
            lo, hi = u - 0.5, u + 0.5
            uA = np.maximum(u1, lo); uB = np.minimum(u2, hi)
            w = (uB > uA) * (h / (2.0 * den12)) * ((uB - u1) ** 2 - (uA - u1) ** 2)
            uA = np.maximum(u2, lo); uB = np.minimum(u3, hi)
            w = w + (uB > uA) * h * (uB - uA)
            uA = np.maximum(u3, lo); uB = np.minimum(u4, hi)
            w = w + (uB > uA) * (h / (2.0 * den34)) * ((uA - u4) ** 2 - (uB - u4) ** 2)
            np.add.at(acc, np.clip(idx.reshape(-1), 0, Nu - 1),
                      img_v * w.reshape(-1))
        out[t] = acc[:Nu]
    return out.astype(np.float32)


# --------------------------------------------------------------------------
# main entry
# --------------------------------------------------------------------------

def kernel(img, theta):
    img = np.asarray(img, dtype=np.float32)
    theta = np.asarray(theta, dtype=np.float32)
    assert img.shape == (Ny, Nx) and theta.shape == (NTHETA,)

    tables = {a: _angle_tables(theta[a]) for a in range(NTHETA)}
    rect_angles = [a for a in range(NTHETA) if tables[a]["q"] is None]
    dev_angles = [a for a in range(NTHETA) if tables[a]["q"] is not None]
    clsX = [a for a in dev_angles if tables[a]["cls"] == 0]
    clsY = [a for a in dev_angles if tables[a]["cls"] == 1]
    assert len(clsX) <= 4 * APC and len(clsY) <= 4 * APC

    # interleave class angles over 4 cores each, pad with repeats
    def assign(lst, ncores):
        groups = [lst[i::ncores] for i in range(ncores)]
        return [g + [g[-1]] * (APC - len(g)) if g else [dev_angles[0]] * APC
                for g in groups]

    coreX = assign(clsX, 4)
    coreY = assign(clsY, 4)
    core_angles = coreX + coreY

    imgT = np.ascontiguousarray(img.T)
    in_maps, metas = [], []
    for ci in range(NCORES):
        layout = img if ci < 4 else imgT
        im, meta = _core_inputs(layout, core_angles[ci], tables)
        in_maps.append(im)
        metas.append(meta)

    nc, _ = _build_program()
    from concourse import bass_utils
    import os
    trace = bool(int(os.environ.get("CT_TRACE", "0")))
    res = bass_utils.run_bass_kernel_spmd(nc, in_maps, core_ids=list(range(NCORES)),
                                          trace=trace)
    _PROGRAM_CACHE["exec_time_ns"] = getattr(res, "exec_time_ns", None)
    _PROGRAM_CACHE["last_results"] = res

    proj = np.zeros((NTHETA, Nu), dtype=np.float64)
    done = set()
    for ci in range(NCORES):
        R = res.results[ci]["r_out"]  # [APC, NCHUNK, VP, MPAD]
        for ai, m in enumerate(metas[ci]):
            a = m["angle"]
            if a in done:
                continue
            done.add(a)
            W = m["W"]
            Mv = W + 3
            for k in range(NCHUNK):
                base = m["bxi_min"] + m["beta0"][k]
                Rk = R[ai, k].astype(np.float64)
                for v in range(VP):
                    n0 = base + v
                    if n0 >= Nu:
                        break
                    hi = min(Mv, Nu - n0)
                    proj[a, n0:n0 + hi] += Rk[v, :hi]

    if rect_angles:
        proj[rect_angles] = _host_project(img, theta[rect_angles])
    return proj.astype(np.float32)



# revision 2
# speedup vs baseline: 2.2954x; 2.2954x over previous
"""CT parallel-beam 2D forward projector on 8 Trainium2 NeuronCores.

v2: low-rank (SVD) factorization of the per-angle tap-weight fields.

For each view angle the 4 trapezoid tap weights U_j (j=0..3) at pixel (y,x)
depend only on g = f_xi(x) + f_eta(y) (fractional parts of the separable
detector coordinate).  The 512x512 field U_j[y,x] = U_j(f_xi[x]+f_eta[y]) is
numerically low rank (<= ~34 at 1e-3 abs err, worst angle), so the host
factors the stacked [512, 4*512] field matrix M = [U0|U1|U2|U3] ~= By @ Bx
(rank J=64, fp16) per angle, and the device reconstructs all four weight
fields with four 512-col PE matmuls per 128-row chunk (~850ns) instead of
~25 vector ops per pixel.

Device pipeline per (angle, 128-row eta-chunk):
  PE  : U fields = ByT^T @ Bx -> PSUM [128, 2048] fp32 (4x 512-col matmuls)
  ACT : drain PSUM -> SBUF bf16 (2x 1024-wide copies), columns interleaved
        host-side as (U0|U1) and (U2|U3) bf16 pairs
  DVE : T_j = U_j * img via 2 packed scalar_tensor_tensor ops into pair
        tiles [128,2048] bf16 (lane-interleaved pairs); S-halves (adjacent
        xi sums for 2-pixel bins) via 2 more packed stt ops
  GPSIMD: 2 indirect_copy gathers move fp32 WORDS (= bf16 pair per field
        duo) through the host-built monotone xi->bin index stream
  PE  : 8 one-hot eta-binning matmuls (4 taps x {512-col, 19-col} PSUM
        pieces, bf16 stride-2 lane views of the gathered pairs, all +1 sign)
  DVE : drain R psum -> SBUF, single per-angle DMA out
Host: tiny anti-diagonal collapse R[v',m] -> proj[n], plus direct numpy path
for the two degenerate axis-aligned angles.

SPMD: one program for all 8 cores. Cores 0-3 process "class X" angles
(|cos| >= sin) on img; cores 4-7 process "class Y" angles on img.T. All
per-angle variation (SVD factors, one-hots, gather indices) is input data.
"""

import numpy as np

Nx = Ny = 512
Nu = 768
NTHETA = 180
HALF_U = (Nu - 1) / 2.0
NCORES = 8
import os as _os
APC = int(_os.environ.get("CT_APC", "23"))   # angles per core
NCHUNK = 4        # eta chunks of 128
MPAD = 528        # gather output width in words (W <= 513, padded, mult 16)
RW = 531          # R output width (512 + 19)
VP = 96           # local v' bins per chunk (128*0.7072 < 91)
ZERO_COL = 1023   # word index of the zero column in each pair buffer
J = int(_os.environ.get("CT_J", "64"))  # SVD rank per angle
B_RECT = 1e-4     # below this min-slope, use the host rect path

_PROGRAM_CACHE = {}


def _np_bf16():
    import ml_dtypes
    return ml_dtypes.bfloat16


# --------------------------------------------------------------------------
# host tables
# --------------------------------------------------------------------------

def _angle_tables(theta_val):
    th = float(theta_val)
    c, s = np.cos(th), np.sin(th)
    ac, asn = abs(c), abs(s)
    A, B = max(ac, asn), min(ac, asn)
    b2 = ac + asn
    cls = 0 if ac >= asn else 1
    a_xi, a_eta = (c, s) if cls == 0 else (s, c)
    z0 = HALF_U - b2 / 2 - 255.5 * (c + s)
    grid = np.arange(512)
    pxi = a_xi * grid + z0
    peta = a_eta * grid
    bxi = np.floor(pxi).astype(np.int64)
    fxi = pxi - bxi
    beta = np.floor(peta).astype(np.int64)
    feta = peta - beta
    q = 1.0 / (2 * A * B) if B > B_RECT else None
    return dict(c=c, s=s, A=A, B=B, b2=b2, q=q, cls=cls,
                bxi=bxi, fxi=fxi, beta=beta, feta=feta)


def _gather_tables(T):
    """xi-binning run-starts and the shared index stream (length MPAD)."""
    bxi = T["bxi"]
    bxi_min = int(bxi.min())
    mloc = bxi - bxi_min
    W = int(mloc.max()) + 1
    xa = np.zeros(W, dtype=np.int64)
    L = np.zeros(W, dtype=np.int64)
    order = np.argsort(mloc, kind="stable")
    sorted_m = mloc[order]
    first = np.searchsorted(sorted_m, np.arange(W), side="left")
    last = np.searchsorted(sorted_m, np.arange(W), side="right")
    for m in range(W):
        idxs = order[first[m]:last[m]]
        n = len(idxs)
        assert 1 <= n <= 2
        xa[m] = idxs.min()
        L[m] = n
        if n == 2:
            assert idxs.max() - idxs.min() == 1

    idx = np.full(MPAD, ZERO_COL, dtype=np.int64)
    msrc = np.arange(0, min(W, MPAD))
    idx[:len(msrc)] = np.where(L[msrc] == 2, 512 + xa[msrc], xa[msrc])
    return dict(bxi_min=bxi_min, W=W, stream=idx)


def _wrap_idx(stream):
    """[MPAD] int -> [128, MPAD//16] uint16 wrapped per 16-partition groups."""
    w = stream.reshape(MPAD // 16, 16).T.astype(np.uint16)
    return np.tile(w, (8, 1))


def _phic(t, A, B):
    q = 1.0 / (2 * A * B)
    r = lambda x: np.square(np.maximum(x, 0.0))
    return q * (r(t) - r(t - B) - r(t - A) + r(t - A - B))


def _angle_factors(T):
    """Rank-J factorization of the stacked tap-weight fields.

    Returns ByT [J, 512] fp16 (chunks of 128 y-rows side by side would be a
    view of this), Bx [J, 2048] fp16 with columns permuted so that the first
    1024 device columns are (U0|U1) lane-interleaved and the last 1024 are
    (U2|U3) lane-interleaved.
    """
    A, B = T["A"], T["B"]
    g = T["feta"][:, None] + T["fxi"][None, :]          # [512, 512]
    Phi1 = _phic(0.5 - g, A, B)
    Phi2 = _phic(1.5 - g, A, B)
    Phi3 = _phic(2.5 - g, A, B)
    U0 = Phi1
    U1 = Phi2 - Phi1
    U2 = Phi3 - Phi2
    U3 = 1.0 - Phi3
    M = np.concatenate([U0, U1, U2, U3], axis=1).astype(np.float64)  # [512,2048]
    MMt = M @ M.T
    w, V = np.linalg.eigh(MMt)
    order = np.argsort(w)[::-1][:J]
    w = np.maximum(w[order], 1e-20)
    V = V[:, order]                                     # [512, J]
    s = np.sqrt(np.sqrt(w))                             # sigma^(1/2)
    By = V * s[None, :]                                 # [512, J]
    Bx = (V / s[None, :]).T @ M                         # [J, 2048]
    # permute Bx columns into the device layout
    perm = np.empty(2048, dtype=np.int64)
    cols = np.arange(512)
    perm[0:1024:2] = 0 * 512 + cols     # U0 -> even lanes of half A
    perm[1:1024:2] = 1 * 512 + cols     # U1 -> odd lanes of half A
    perm[1024:2048:2] = 2 * 512 + cols  # U2 -> even lanes of half B
    perm[1025:2048:2] = 3 * 512 + cols  # U3 -> odd lanes of half B
    Bx = Bx[:, perm]
    return By.astype(np.float16), Bx.astype(np.float16)


def _core_inputs(img_layout, angle_list, tables):
    """Build the input map for one core. img_layout: [512,512] f32 [eta,xi]."""
    bf16 = _np_bf16()
    A_ = APC
    img = np.ascontiguousarray(img_layout).astype(np.float32)
    img2 = np.repeat(img.reshape(NCHUNK, 128, 512), 2, axis=2)  # [4,128,1024]

    ByT_t = np.zeros((A_, J, 512), dtype=np.float16)
    Bx_t = np.zeros((A_, J, 2048), dtype=np.float16)
    oh_t = np.zeros((A_, 128, NCHUNK * VP), dtype=np.float32)
    idx_t = np.zeros((A_, 128, MPAD // 16), dtype=np.uint16)
    meta = []
    for ai, a in enumerate(angle_list):
        T = tables[a]
        G = _gather_tables(T)
        By, Bx = _angle_factors(T)
        ByT_t[ai] = By.T
        Bx_t[ai] = Bx
        beta = T["beta"]
        beta0 = []
        for k in range(NCHUNK):
            sl = slice(k * 128, (k + 1) * 128)
            vloc = beta[sl] - beta[sl].min()
            assert vloc.min() >= 0 and vloc.max() < VP
            oh_t[ai, np.arange(128), k * VP + vloc] = 1.0
            beta0.append(int(beta[sl].min()))
        idx_t[ai] = _wrap_idx(G["stream"])
        meta.append(dict(angle=a, bxi_min=G["bxi_min"], W=G["W"], beta0=beta0))
    in_map = {
        "img2_t": img2.astype(bf16),
        "ByT_t": ByT_t,
        "Bx_t": Bx_t,
        "oh_t": oh_t.astype(bf16),
        "idx_t": idx_t,
    }
    return in_map, meta


# --------------------------------------------------------------------------
# the bass program (identical for all cores)
# --------------------------------------------------------------------------

def _build_program():
    if "nc" in _PROGRAM_CACHE:
        return _PROGRAM_CACHE["nc"]

    import concourse.bass as bass
    import concourse.tile as tile
    from concourse import bacc, mybir
    from contextlib import ExitStack

    dt = mybir.dt
    ALU = mybir.AluOpType

    nc = bacc.Bacc("TRN2", target_bir_lowering=False, debug=False,
                   num_devices=NCORES)

    img2_t = nc.dram_tensor("img2_t", [NCHUNK, 128, 1024], dt.bfloat16,
                            kind="ExternalInput").ap()
    ByT_t = nc.dram_tensor("ByT_t", [APC, J, 512], dt.float16,
                           kind="ExternalInput").ap()
    Bx_t = nc.dram_tensor("Bx_t", [APC, J, 2048], dt.float16,
                          kind="ExternalInput").ap()
    oh_t = nc.dram_tensor("oh_t", [APC, 128, NCHUNK * VP], dt.bfloat16,
                          kind="ExternalInput").ap()
    idx_t = nc.dram_tensor("idx_t", [APC, 128, MPAD // 16], dt.uint16,
                           kind="ExternalInput").ap()
    r_out = nc.dram_tensor("r_out", [APC, VP, NCHUNK, RW], dt.float32,
                           kind="ExternalOutput").ap()

    with tile.TileContext(nc) as tc, ExitStack() as ctx:
        img_pool = ctx.enter_context(tc.tile_pool(name="img", bufs=1))
        pair_pool = ctx.enter_context(tc.tile_pool(name="pairs", bufs=1))
        tab_pool = ctx.enter_context(tc.tile_pool(name="tabs", bufs=2))
        phi_pool = ctx.enter_context(tc.tile_pool(name="phi", bufs=2))
        g_pool = ctx.enter_context(tc.tile_pool(name="gath", bufs=2))
        o_pool = ctx.enter_context(tc.tile_pool(name="outs", bufs=2))
        psf_pool = ctx.enter_context(tc.tile_pool(name="psumF", bufs=2,
                                                  space="PSUM"))
        psr_pool = ctx.enter_context(tc.tile_pool(name="psumR", bufs=2,
                                                  space="PSUM"))

        # resident image chunks (pair-duplicated bf16) and pair work tiles
        img2_ch = []
        pairs = []
        for k in range(NCHUNK):
            t = img_pool.tile([128, 1024], dt.bfloat16, tag=f"img2c{k}")
            nc.sync.dma_start(t[:], img2_t[k])
            img2_ch.append(t)
            pr = []
            for h in range(2):
                p = pair_pool.tile([128, 2048], dt.bfloat16, tag=f"pair{k}_{h}")
                nc.vector.memset(p[:, 2046:2048], 0.0)  # zero word 1023
                pr.append(p)
            pairs.append(pr)

        for ai in range(APC):
            idxt = tab_pool.tile([128, MPAD // 16], dt.uint16, tag="idx")
            nc.sync.dma_start(idxt[:], idx_t[ai])
            bx = tab_pool.tile([J, 2048], dt.float16, tag="bx")
            nc.sync.dma_start(bx[:], Bx_t[ai])
            byt = tab_pool.tile([J, 512], dt.float16, tag="byt")
            nc.sync.dma_start(byt[:], ByT_t[ai])
            oht = tab_pool.tile([128, NCHUNK * VP], dt.bfloat16, tag="oh")
            nc.sync.dma_start(oht[:], oh_t[ai])

            rout = o_pool.tile([VP, NCHUNK * RW], dt.float32, tag="rout")

            for k in range(NCHUNK):
                byk = byt[:, 128 * k:128 * (k + 1)]
                ohk = oht[:, VP * k:VP * (k + 1)]
                pairA, pairB = pairs[k]

                # U fields: psF halves [128,1024] f32, 2x 512-col matmuls each
                phis = []
                for h in range(2):
                    psF = psf_pool.tile([128, 1024], dt.float32, tag="psF")
                    nc.tensor.matmul(psF[:, 0:512], byk,
                                     bx[:, 1024 * h:1024 * h + 512],
                                     start=True, stop=True)
                    nc.tensor.matmul(psF[:, 512:1024], byk,
                                     bx[:, 1024 * h + 512:1024 * (h + 1)],
                                     start=True, stop=True)
                    phi = phi_pool.tile([128, 1024], dt.bfloat16, tag=f"phi{h}")
                    nc.scalar.copy(phi[:], psF[:])   # ACT drain, bf16 out
                    phis.append(phi)

                # T_j = U_j * img (packed bf16 pairs), then S-halves
                for h, pt in enumerate((pairA, pairB)):
                    nc.vector.scalar_tensor_tensor(
                        pt[:, 0:1024], phis[h][:], 1.0, img2_ch[k][:],
                        ALU.mult, ALU.mult)
                    nc.vector.scalar_tensor_tensor(
                        pt[:, 1024:2046], pt[:, 0:1022], 0.0, pt[:, 2:1024],
                        ALU.add, ALU.add)

                # gathers: fp32-word views move bf16 pairs
                gts = []
                for h, pt in enumerate((pairA, pairB)):
                    gt = g_pool.tile([128, MPAD], dt.float32, tag=f"g{h}")
                    nc.gpsimd.indirect_copy(gt[:], pt[:].bitcast(dt.float32),
                                            idxt[:], True)
                    gts.append(gt)

                # one-hot eta-binning; tap shift s as PSUM column offset
                ps = psr_pool.tile([VP, 544], dt.float32, tag="ps")
                lanes = []
                for h in range(2):
                    bf = gts[h][:].bitcast(dt.bfloat16)
                    v = bf.rearrange("p (w l) -> p w l", l=2)
                    lanes.append(v[:, :, 0])
                    lanes.append(v[:, :, 1])
                # ps[:, 0:512] covers m in [0,512); ps[:, 512:531] covers
                # [512,531). First writer of each piece must span it fully.
                for s in (0, 1, 2, 3):
                    nc.tensor.matmul(ps[:, s:512], ohk, lanes[s][:, 0:512 - s],
                                     start=(s == 0), stop=(s == 3))
                for s in (3, 2, 1, 0):
                    nc.tensor.matmul(ps[:, 512:528 + s], ohk,
                                     lanes[s][:, 512 - s:528],
                                     start=(s == 3), stop=(s == 0))

                nc.vector.tensor_copy(rout[:, RW * k:RW * k + 512],
                                      ps[:, 0:512])
                nc.vector.tensor_copy(rout[:, RW * k + 512:RW * (k + 1)],
                                      ps[:, 512:531])

            nc.sync.dma_start(r_out[ai], rout[:])

    nc.compile()
    _PROGRAM_CACHE["nc"] = nc
    return nc


# --------------------------------------------------------------------------
# host-side rect path (degenerate angles) — numpy port of the reference
# --------------------------------------------------------------------------

def _host_project(img, theta_vals):
    y = (np.arange(Ny) - (Ny - 1) / 2.0)
    x = (np.arange(Nx) - (Nx - 1) / 2.0)
    y2d, x2d = np.meshgrid(y, x, indexing="ij")
    img_v = img.reshape(-1).astype(np.float64)
    out = np.zeros((len(theta_vals), Nu), dtype=np.float64)
    K = 4
    for t, th in enumerate(theta_vals):
        th = float(th)
        cos_t, sin_t = np.cos(th), np.sin(th)
        ac, asn = abs(cos_t), abs(sin_t)
        h = min(1.0 / ac if ac > 0 else np.inf, 1.0 / asn if asn > 0 else np.inf)
        b1 = abs(asn - ac)
        b2 = abs(asn + ac)
        u0 = x2d * cos_t + y2d * sin_t
        u1 = u0 - b2 / 2
        u2 = u0 - b1 / 2
        u3 = u0 + b1 / 2
        u4 = u0 + b2 / 2
        base = np.floor(u1 + HALF_U).astype(np.int64)
        den12 = (u2 - u1) + (u1 == u2)
        den34 = (u4 - u3) + (u3 == u4)
        acc = np.zeros(Nu + 8, dtype=np.float64)
        for k in range(K):
            idx = base + k
            u = idx - HALF_U
            lo, hi = u - 0.5, u + 0.5
            uA = np.maximum(u1, lo); uB = np.minimum(u2, hi)
            w = (uB > uA) * (h / (2.0 * den12)) * ((uB - u1) ** 2 - (uA - u1) ** 2)
            uA = np.maximum(u2, lo); uB = np.minimum(u3, hi)
            w = w + (uB > uA) * h * (uB - uA)
            uA = np.maximum(u3, lo); uB = np.minimum(u4, hi)
            w = w + (uB > uA) * (h / (2.0 * den34)) * ((uA - u4) ** 2 - (uB - u4) ** 2)
            np.add.at(acc, np.clip(idx.reshape(-1), 0, Nu - 1),
                      img_v * w.reshape(-1))
        out[t] = acc[:Nu]
    return out.astype(np.float32)


# --------------------------------------------------------------------------
# main entry
# --------------------------------------------------------------------------

def kernel(img, theta):
    img = np.asarray(img, dtype=np.float32)
    theta = np.asarray(theta, dtype=np.float32)
    assert img.shape == (Ny, Nx) and theta.shape == (NTHETA,)

    tables = {a: _angle_tables(theta[a]) for a in range(NTHETA)}
    rect_angles = [a for a in range(NTHETA) if tables[a]["q"] is None]
    dev_angles = [a for a in range(NTHETA) if tables[a]["q"] is not None]
    clsX = [a for a in dev_angles if tables[a]["cls"] == 0]
    clsY = [a for a in dev_angles if tables[a]["cls"] == 1]
    assert len(clsX) <= 4 * APC and len(clsY) <= 4 * APC

    def assign(lst, ncores):
        groups = [lst[i::ncores] for i in range(ncores)]
        return [g + [g[-1]] * (APC - len(g)) if g else [dev_angles[0]] * APC
                for g in groups]

    core_angles = assign(clsX, 4) + assign(clsY, 4)

    imgT = np.ascontiguousarray(img.T)
    in_maps, metas = [], []
    for ci in range(NCORES):
        layout = img if ci < 4 else imgT
        im, meta = _core_inputs(layout, core_angles[ci], tables)
        in_maps.append(im)
        metas.append(meta)

    nc = _build_program()
    from concourse import bass_utils
    import os
    trace = bool(int(os.environ.get("CT_TRACE", "0")))
    res = bass_utils.run_bass_kernel_spmd(nc, in_maps,
                                          core_ids=list(range(NCORES)),
                                          trace=trace)
    _PROGRAM_CACHE["exec_time_ns"] = getattr(res, "exec_time_ns", None)
    _PROGRAM_CACHE["last_results"] = res

    proj = np.zeros((NTHETA, Nu), dtype=np.float64)
    done = set()
    for ci in range(NCORES):
        R = res.results[ci]["r_out"]  # [APC, VP, NCHUNK, RW]
        for ai, m in enumerate(metas[ci]):
            a = m["angle"]
            if a in done:
                continue
            done.add(a)
            Mv = m["W"] + 3
            for k in range(NCHUNK):
                base = m["bxi_min"] + m["beta0"][k]
                Rk = R[:, ai] if False else R[ai, :, k]  # [VP, RW]
                Rk = Rk.astype(np.float64)
                for v in range(VP):
                    n0 = base + v
                    if n0 >= Nu:
                        break
                    hi = min(Mv, Nu - n0)
                    proj[a, n0:n0 + hi] += Rk[v, :hi]

    if rect_angles:
        proj[rect_angles] = _host_project(img, theta[rect_angles])
    return proj.astype(np.float32)


# revision 4
# speedup vs baseline: 3.5172x; 1.5323x over previous
"""CT parallel-beam 2D forward projector on 8 Trainium2 NeuronCores.

v3: low-rank (SVD) factorization of the per-angle cumulative-weight fields.

For each view angle the trapezoid footprint weights at pixel (y,x) depend
only on g = f_xi(x) + f_eta(y) (fractional parts of the separable detector
coordinate).  The cumulative fields Phi_i[y,x] = Phic(i-0.5-g) (i=1,2,3)
are numerically low rank (<= ~36 at 5e-4 abs err, worst angle), so the host
factors the stacked [512, 3*512] field matrix M = [Phi1|Phi2|Phi3] ~= By@Bx
(rank J=64, fp16) per angle, and the device reconstructs the weight fields
with three 512-col PE matmuls per 128-row chunk (~640ns) instead of ~25
vector ops per pixel.

Device pipeline per (angle, 128-row eta-chunk):
  PE  : Phi fields = ByT^T @ Bx -> PSUM fp32 (3x 512-col matmuls); half A
        holds (Phi1|Phi2) column-interleaved (host-permuted Bx), half B Phi3
  ACT : drain PSUM -> SBUF bf16
  DVE : T_i = Phi_i * img via packed bf16 tensor_tensor (2x mode) into pair
        tiles [128,2048] bf16 = lane-interleaved (T1|T2) and (T3|img) (img
        lanes persistent); S-halves (adjacent xi sums for 2-pixel bins) via
        packed tensor_tensor adds
  GPSIMD: 2 indirect_copy gathers move fp32 WORDS (= a bf16 field pair)
        through the host-built monotone xi->bin index stream
  PE  : one-hot eta-binning, 7 signed (field, tap-shift) instances as PSUM
        column offsets, bf16 stride-2 lane views of the gathered pairs
  ACT/GPSIMD: drain R psum -> SBUF, single per-angle DMA out
Host: tiny anti-diagonal collapse R[v',m] -> proj[n], plus direct numpy path
for the two degenerate axis-aligned angles.

SPMD: one program for all 8 cores. Cores 0-3 process "class X" angles
(|cos| >= sin) on img; cores 4-7 process "class Y" angles on img.T. All
per-angle variation (SVD factors, one-hots, gather indices) is input data.
"""

import numpy as np

Nx = Ny = 512
Nu = 768
NTHETA = 180
HALF_U = (Nu - 1) / 2.0
NCORES = 8
import os as _os
APC = int(_os.environ.get("CT_APC", "23"))   # angles per core
NCHUNK = 4        # eta chunks of 128
MPAD = 528        # gather output width in words (W <= 513, padded, mult 16)
RW = 531          # R output width (512 + 19)
VP = 96           # local v' bins per chunk (128*0.7072 < 91)
NV = NCHUNK * VP
ZERO_COL = 1023   # word index of the zero column in each pair buffer
J = int(_os.environ.get("CT_J", "64"))  # SVD rank per angle
B_RECT = 1e-4     # below this min-slope, use the host rect path

_PROGRAM_CACHE = {}


def _np_bf16():
    import ml_dtypes
    return ml_dtypes.bfloat16


# --------------------------------------------------------------------------
# host tables
# --------------------------------------------------------------------------

def _angle_tables(theta_val):
    th = float(theta_val)
    c, s = np.cos(th), np.sin(th)
    ac, asn = abs(c), abs(s)
    A, B = max(ac, asn), min(ac, asn)
    b2 = ac + asn
    cls = 0 if ac >= asn else 1
    a_xi, a_eta = (c, s) if cls == 0 else (s, c)
    z0 = HALF_U - b2 / 2 - 255.5 * (c + s)
    grid = np.arange(512)
    pxi = a_xi * grid + z0
    peta = a_eta * grid
    bxi = np.floor(pxi).astype(np.int64)
    fxi = pxi - bxi
    beta = np.floor(peta).astype(np.int64)
    feta = peta - beta
    q = 1.0 / (2 * A * B) if B > B_RECT else None
    return dict(c=c, s=s, A=A, B=B, b2=b2, q=q, cls=cls,
                bxi=bxi, fxi=fxi, beta=beta, feta=feta)


def _gather_tables(T):
    """xi-binning run-starts and the shared index stream (length MPAD)."""
    bxi = T["bxi"]
    bxi_min = int(bxi.min())
    mloc = bxi - bxi_min
    W = int(mloc.max()) + 1
    xa = np.zeros(W, dtype=np.int64)
    L = np.zeros(W, dtype=np.int64)
    order = np.argsort(mloc, kind="stable")
    sorted_m = mloc[order]
    first = np.searchsorted(sorted_m, np.arange(W), side="left")
    last = np.searchsorted(sorted_m, np.arange(W), side="right")
    for m in range(W):
        idxs = order[first[m]:last[m]]
        n = len(idxs)
        assert 1 <= n <= 2
        xa[m] = idxs.min()
        L[m] = n
        if n == 2:
            assert idxs.max() - idxs.min() == 1

    idx = np.full(MPAD, ZERO_COL, dtype=np.int64)
    msrc = np.arange(0, min(W, MPAD))
    idx[:len(msrc)] = np.where(L[msrc] == 2, 512 + xa[msrc], xa[msrc])
    return dict(bxi_min=bxi_min, W=W, stream=idx)


def _wrap_idx(stream):
    """[MPAD] int -> [128, MPAD//16] uint16 wrapped per 16-partition groups."""
    w = stream.reshape(MPAD // 16, 16).T.astype(np.uint16)
    return np.tile(w, (8, 1))


def _phic(t, A, B):
    q = 1.0 / (2 * A * B)
    r = lambda x: np.square(np.maximum(x, 0.0))
    return q * (r(t) - r(t - B) - r(t - A) + r(t - A - B))


def _angle_factors(T):
    """Rank-J factorization of the stacked cumulative fields.

    Returns By [512, J] fp16 and Bx [J, 1536] fp16 with columns permuted so
    device cols 0:1024 are (Phi1|Phi2) lane-interleaved, 1024:1536 Phi3.
    """
    A, B = T["A"], T["B"]
    g = T["feta"][:, None] + T["fxi"][None, :]          # [512, 512]
    Phi1 = _phic(0.5 - g, A, B)
    Phi2 = _phic(1.5 - g, A, B)
    Phi3 = _phic(2.5 - g, A, B)
    M = np.concatenate([Phi1, Phi2, Phi3], axis=1)      # [512, 1536]
    MMt = M @ M.T
    w, V = np.linalg.eigh(MMt)
    order = np.argsort(w)[::-1][:J]
    w = np.maximum(w[order], 1e-20)
    V = V[:, order]                                     # [512, J]
    s = np.sqrt(np.sqrt(w))                             # sigma^(1/2)
    By = V * s[None, :]
    Bx = (V / s[None, :]).T @ M                         # [J, 1536]
    perm = np.empty(1536, dtype=np.int64)
    cols = np.arange(512)
    perm[0:1024:2] = 0 * 512 + cols
    perm[1:1024:2] = 1 * 512 + cols
    perm[1024:1536] = 2 * 512 + cols
    Bx = Bx[:, perm]
    return By.astype(np.float16), Bx.astype(np.float16)


def _core_inputs(img_layout, angle_list, tables):
    """Build the input map for one core. img_layout: [512,512] f32 [eta,xi]."""
    bf16 = _np_bf16()
    A_ = APC
    img = np.ascontiguousarray(img_layout).astype(np.float32)
    imgc = img.reshape(NCHUNK, 128, 512)
    img2 = np.repeat(imgc, 2, axis=2)                   # [4,128,1024]
    img_b = imgc.astype(bf16).astype(np.float32)
    # pairB initial content: odd F lanes = img, odd S lanes = S(img)
    pairB0 = np.zeros((NCHUNK, 128, 2048), dtype=np.float32)
    pairB0[:, :, 1:1024:2] = img_b
    pairB0[:, :, 1025:2046:2] = img_b[:, :, :-1] + img_b[:, :, 1:]

    ByT_t = np.zeros((A_, J, 512), dtype=np.float16)
    Bx_t = np.zeros((A_, J, 1536), dtype=np.float16)
    oh_t = np.zeros((A_, 128, 2 * NV), dtype=np.float32)
    idx_t = np.zeros((A_, 128, MPAD // 16), dtype=np.uint16)
    meta = []
    for ai, a in enumerate(angle_list):
        T = tables[a]
        G = _gather_tables(T)
        By, Bx = _angle_factors(T)
        ByT_t[ai] = By.T
        Bx_t[ai] = Bx
        beta = T["beta"]
        beta0 = []
        for k in range(NCHUNK):
            sl = slice(k * 128, (k + 1) * 128)
            vloc = beta[sl] - beta[sl].min()
            assert vloc.min() >= 0 and vloc.max() < VP
            oh_t[ai, np.arange(128), k * VP + vloc] = 1.0
            oh_t[ai, np.arange(128), NV + k * VP + vloc] = -1.0
            beta0.append(int(beta[sl].min()))
        idx_t[ai] = _wrap_idx(G["stream"])
        meta.append(dict(angle=a, bxi_min=G["bxi_min"], W=G["W"], beta0=beta0))
    in_map = {
        "img2_t": img2.astype(bf16),
        "imgc_t": imgc.astype(bf16),
        "pairB0_t": pairB0.astype(bf16),
        "ByT_t": ByT_t,
        "Bx_t": Bx_t,
        "oh_t": oh_t.astype(bf16),
        "idx_t": idx_t,
    }
    return in_map, meta


# --------------------------------------------------------------------------
# the bass program (identical for all cores)
# --------------------------------------------------------------------------

def _build_program():
    if "nc" in _PROGRAM_CACHE:
        return _PROGRAM_CACHE["nc"]

    import concourse.bass as bass
    import concourse.tile as tile
    from concourse import bacc, mybir
    from contextlib import ExitStack

    dt = mybir.dt
    ALU = mybir.AluOpType

    nc = bacc.Bacc("TRN2", target_bir_lowering=False, debug=False,
                   num_devices=NCORES)

    img2_t = nc.dram_tensor("img2_t", [NCHUNK, 128, 1024], dt.bfloat16,
                            kind="ExternalInput").ap()
    imgc_t = nc.dram_tensor("imgc_t", [NCHUNK, 128, 512], dt.bfloat16,
                            kind="ExternalInput").ap()
    pairB0_t = nc.dram_tensor("pairB0_t", [NCHUNK, 128, 2048], dt.bfloat16,
                              kind="ExternalInput").ap()
    ByT_t = nc.dram_tensor("ByT_t", [APC, J, 512], dt.float16,
                           kind="ExternalInput").ap()
    Bx_t = nc.dram_tensor("Bx_t", [APC, J, 1536], dt.float16,
                          kind="ExternalInput").ap()
    oh_t = nc.dram_tensor("oh_t", [APC, 128, 2 * NV], dt.bfloat16,
                          kind="ExternalInput").ap()
    idx_t = nc.dram_tensor("idx_t", [APC, 128, MPAD // 16], dt.uint16,
                           kind="ExternalInput").ap()
    r_out = nc.dram_tensor("r_out", [APC, VP, NCHUNK, RW], dt.float32,
                           kind="ExternalOutput").ap()

    # (field, psum column shift, sign); order below ensures the first writer
    # of each PSUM piece covers its full written range
    instances = [(0, 0, +1), (3, 3, +1), (1, 1, +1), (2, 2, +1),
                 (0, 1, -1), (1, 2, -1), (2, 3, -1)]

    with tile.TileContext(nc) as tc, ExitStack() as ctx:
        img_pool = ctx.enter_context(tc.tile_pool(name="img", bufs=1))
        pair_pool = ctx.enter_context(tc.tile_pool(name="pairs", bufs=1))
        tab_pool = ctx.enter_context(tc.tile_pool(name="tabs", bufs=2))
        phi_pool = ctx.enter_context(tc.tile_pool(name="phi", bufs=2))
        g_pool = ctx.enter_context(tc.tile_pool(name="gath", bufs=2))
        o_pool = ctx.enter_context(tc.tile_pool(name="outs", bufs=2))
        psfa_pool = ctx.enter_context(tc.tile_pool(name="psumFa", bufs=2,
                                                   space="PSUM"))
        psfb_pool = ctx.enter_context(tc.tile_pool(name="psumFb", bufs=2,
                                                   space="PSUM"))
        psr_pool = ctx.enter_context(tc.tile_pool(name="psumR", bufs=1,
                                                  space="PSUM"))

        img2_ch, imgc_ch, pairsA, pairsB = [], [], [], []
        for k in range(NCHUNK):
            t = img_pool.tile([128, 1024], dt.bfloat16, tag=f"img2c{k}")
            nc.sync.dma_start(t[:], img2_t[k])
            img2_ch.append(t)
            t = img_pool.tile([128, 512], dt.bfloat16, tag=f"imgcc{k}")
            nc.sync.dma_start(t[:], imgc_t[k])
            imgc_ch.append(t)
            pa = pair_pool.tile([128, 2048], dt.bfloat16, tag=f"pairA{k}")
            nc.vector.memset(pa[:, 2046:2048], 0.0)
            pairsA.append(pa)
            pb = pair_pool.tile([128, 2048], dt.bfloat16, tag=f"pairB{k}")
            nc.sync.dma_start(pb[:], pairB0_t[k])
            pairsB.append(pb)

        for ai in range(APC):
            idxt = tab_pool.tile([128, MPAD // 16], dt.uint16, tag="idx")
            nc.sync.dma_start(idxt[:], idx_t[ai])
            bx = tab_pool.tile([J, 1536], dt.float16, tag="bx")
            nc.sync.dma_start(bx[:], Bx_t[ai])
            byt = tab_pool.tile([J, 512], dt.float16, tag="byt")
            nc.sync.dma_start(byt[:], ByT_t[ai])
            oht = tab_pool.tile([128, 2 * NV], dt.bfloat16, tag="oh")
            nc.sync.dma_start(oht[:], oh_t[ai])

            rout = o_pool.tile([VP, NCHUNK * RW], dt.float32, tag="rout")

            for k in range(NCHUNK):
                byk = byt[:, 128 * k:128 * (k + 1)]
                ohk = oht[:, VP * k:VP * (k + 1)]
                ohnk = oht[:, NV + VP * k:NV + VP * (k + 1)]
                pairA, pairB = pairsA[k], pairsB[k]

                psFa = psfa_pool.tile([128, 1024], dt.float32, tag="psFa")
                nc.tensor.matmul(psFa[:, 0:512], byk, bx[:, 0:512],
                                 start=True, stop=True)
                nc.tensor.matmul(psFa[:, 512:1024], byk, bx[:, 512:1024],
                                 start=True, stop=True)
                psFb = psfb_pool.tile([128, 512], dt.float32, tag="psFb")
                nc.tensor.matmul(psFb[:], byk, bx[:, 1024:1536],
                                 start=True, stop=True)

                phiA = phi_pool.tile([128, 1024], dt.bfloat16, tag="phiA")
                nc.scalar.copy(phiA[:], psFa[:])
                phiB = phi_pool.tile([128, 512], dt.bfloat16, tag="phiB")
                nc.scalar.copy(phiB[:], psFb[:])

                # T fields (packed bf16 TT, 2x mode) and S-halves
                nc.vector.tensor_tensor(pairA[:, 0:1024], phiA[:],
                                        img2_ch[k][:], ALU.mult)
                pairB_evenF = pairB[:].rearrange(
                    "p (w l) -> p w l", l=2)[:, 0:512, 0]
                nc.vector.tensor_tensor(pairB_evenF, phiB[:],
                                        imgc_ch[k][:], ALU.mult)
                nc.vector.tensor_tensor(pairA[:, 1024:2046], pairA[:, 0:1022],
                                        pairA[:, 2:1024], ALU.add)
                nc.vector.tensor_tensor(pairB[:, 1024:2046], pairB[:, 0:1022],
                                        pairB[:, 2:1024], ALU.add)

                # gathers: fp32-word views move bf16 field pairs
                gts = []
                for h, pt in enumerate((pairA, pairB)):
                    gt = g_pool.tile([128, MPAD], dt.float32, tag=f"g{h}")
                    nc.gpsimd.indirect_copy(gt[:], pt[:].bitcast(dt.float32),
                                            idxt[:], True)
                    gts.append(gt)

                lanes = []
                for h in range(2):
                    v = gts[h][:].bitcast(dt.bfloat16).rearrange(
                        "p (w l) -> p w l", l=2)
                    lanes.append(v[:, :, 0])
                    lanes.append(v[:, :, 1])

                # one-hot eta-binning; tap shift s as PSUM column offset
                ps = psr_pool.tile([VP, 544], dt.float32, tag="ps")
                mms = []
                for f, s, sgn in instances:
                    lhs = ohk if sgn > 0 else ohnk
                    mms.append((0, ps[:, s:512], lhs, lanes[f][:, 0:512 - s]))
                    mms.append((1, ps[:, 512:528 + s], lhs,
                                lanes[f][:, 512 - s:528]))
                order = [0, 3, 1, 2] + list(range(4, 14))
                started = {0: False, 1: False}
                last_pos = {t: max(p for p, m in enumerate(order)
                                   if mms[m][0] == t) for t in (0, 1)}
                for pos, mi in enumerate(order):
                    tid, out_ap, lhs, rhs_ap = mms[mi]
                    nc.tensor.matmul(out_ap, lhs, rhs_ap,
                                     start=not started[tid],
                                     stop=(pos == last_pos[tid]))
                    started[tid] = True

                nc.scalar.copy(rout[:, RW * k:RW * k + 512], ps[:, 0:512])
                nc.scalar.copy(rout[:, RW * k + 512:RW * (k + 1)],
                               ps[:, 512:531])

            nc.sync.dma_start(r_out[ai], rout[:])

    nc.compile()
    _PROGRAM_CACHE["nc"] = nc
    return nc


# --------------------------------------------------------------------------
# host-side rect path (degenerate angles) — numpy port of the reference
# --------------------------------------------------------------------------

def _host_project(img, theta_vals):
    y = (np.arange(Ny) - (Ny - 1) / 2.0)
    x = (np.arange(Nx) - (Nx - 1) / 2.0)
    y2d, x2d = np.meshgrid(y, x, indexing="ij")
    img_v = img.reshape(-1).astype(np.float64)
    out = np.zeros((len(theta_vals), Nu), dtype=np.float64)
    K = 4
    for t, th in enumerate(theta_vals):
        th = float(th)
        cos_t, sin_t = np.cos(th), np.sin(th)
        ac, asn = abs(cos_t), abs(sin_t)
        h = min(1.0 / ac if ac > 0 else np.inf, 1.0 / asn if asn > 0 else np.inf)
        b1 = abs(asn - ac)
        b2 = abs(asn + ac)
        u0 = x2d * cos_t + y2d * sin_t
        u1 = u0 - b2 / 2
        u2 = u0 - b1 / 2
        u3 = u0 + b1 / 2
        u4 = u0 + b2 / 2
        base = np.floor(u1 + HALF_U).astype(np.int64)
        den12 = (u2 - u1) + (u1 == u2)
        den34 = (u4 - u3) + (u3 == u4)
        acc = np.zeros(Nu + 8, dtype=np.float64)
        for k in range(K):
            idx = base + k
            u = idx - HALF_U
            lo, hi = u - 0.5, u + 0.5
            uA = np.maximum(u1, lo); uB = np.minimum(u2, hi)
            w = (uB > uA) * (h / (2.0 * den12)) * ((uB - u1) ** 2 - (uA - u1) ** 2)
            uA = np.maximum(u2, lo); uB = np.minimum(u3, hi)
            w = w + (uB > uA) * h * (uB - uA)
            uA = np.maximum(u3, lo); uB = np.minimum(u4, hi)
            w = w + (uB > uA) * (h / (2.0 * den34)) * ((uA - u4) ** 2 - (uB - u4) ** 2)
            np.add.at(acc, np.clip(idx.reshape(-1), 0, Nu - 1),
                      img_v * w.reshape(-1))
        out[t] = acc[:Nu]
    return out.astype(np.float32)


# --------------------------------------------------------------------------
# main entry
# --------------------------------------------------------------------------

def kernel(img, theta):
    img = np.asarray(img, dtype=np.float32)
    theta = np.asarray(theta, dtype=np.float32)
    assert img.shape == (Ny, Nx) and theta.shape == (NTHETA,)

    tables = {a: _angle_tables(theta[a]) for a in range(NTHETA)}
    rect_angles = [a for a in range(NTHETA) if tables[a]["q"] is None]
    dev_angles = [a for a in range(NTHETA) if tables[a]["q"] is not None]
    clsX = [a for a in dev_angles if tables[a]["cls"] == 0]
    clsY = [a for a in dev_angles if tables[a]["cls"] == 1]
    assert len(clsX) <= 4 * APC and len(clsY) <= 4 * APC

    def assign(lst, ncores):
        groups = [lst[i::ncores] for i in range(ncores)]
        return [g + [g[-1]] * (APC - len(g)) if g else [dev_angles[0]] * APC
                for g in groups]

    core_angles = assign(clsX, 4) + assign(clsY, 4)

    imgT = np.ascontiguousarray(img.T)
    in_maps, metas = [], []
    for ci in range(NCORES):
        layout = img if ci < 4 else imgT
        im, meta = _core_inputs(layout, core_angles[ci], tables)
        in_maps.append(im)
        metas.append(meta)

    nc = _build_program()
    from concourse import bass_utils
    import os
    trace = bool(int(os.environ.get("CT_TRACE", "0")))
    res = bass_utils.run_bass_kernel_spmd(nc, in_maps,
                                          core_ids=list(range(NCORES)),
                                          trace=trace)
    _PROGRAM_CACHE["exec_time_ns"] = getattr(res, "exec_time_ns", None)
    _PROGRAM_CACHE["last_results"] = res

    proj = np.zeros((NTHETA, Nu), dtype=np.float64)
    done = set()
    for ci in range(NCORES):
        R = res.results[ci]["r_out"]  # [APC, VP, NCHUNK, RW]
        for ai, m in enumerate(metas[ci]):
            a = m["angle"]
            if a in done:
                continue
            done.add(a)
            Mv = m["W"] + 3
            for k in range(NCHUNK):
                base = m["bxi_min"] + m["beta0"][k]
                Rk = R[ai, :, k].astype(np.float64)
                for v in range(VP):
                    n0 = base + v
                    if n0 >= Nu:
                        break
                    hi = min(Mv, Nu - n0)
                    proj[a, n0:n0 + hi] += Rk[v, :hi]

    if rect_angles:
        proj[rect_angles] = _host_project(img, theta[rect_angles])
    return proj.astype(np.float32)
